# revision 21
# baseline (speedup 1.0000x reference)
"""End2EndPoseLoss on 8 Trainium2 NeuronCores.

Data-parallel over batch: each core handles B_LOC=2 samples.

Heavy part (per core): row-chunk sums over [680, 4096] pred/gt heatmaps.
Uses the identity (pred-gt)^2 * step(gt>thresh) == ((pred-gt)*step)^2 so
the per-chunk dependency graph is forward-only DVE -> ACT:
  DVE: d = p - g            (tensor_tensor)
  DVE: m = step(g>.2) * d   (scalar_tensor_tensor)
  ACT: Square(d)  + row-accumulate -> sums_sq column
  ACT: Square(m)  + row-accumulate -> sums_st column
Raw per-(row-tile, chunk) sums [128, 12] are DMA'd out; the host applies
the 0/1 sample mask per row and the weighted combination (the scalar
"all-reduce" across cores).

Small losses (count CE over [2,21], conf focal over [2,20]) run on-device
too, issued before the heavy loop so their ACT work hides in the DMA
ramp; the two Ln ops run at the end (one activation-table switch).
"""

import sys
import types
import numpy as np

import concourse.bacc as bacc
import concourse.bass as bass  # noqa: F401
import concourse.mybir as mybir
import concourse.tile as tile
from concourse import bass_utils

# Problem constants (hardcoded per contract).
B, P, K, H, W = 16, 20, 17, 64, 64
N_CORES = 8
B_LOC = B // N_CORES            # 2
ROWS = B_LOC * P * K            # 680
COLS = H * W                    # 4096
FULL_TILES = ROWS // 128         # 5 row-tiles of 128 full rows
REM = ROWS - FULL_TILES * 128    # 40 leftover rows -> folded [80, 2048]
NACC = FULL_TILES + 1            # 6 accumulator columns

PEAK_THRESH = 0.2
PEAK_WEIGHT = 5.0
FOCAL_GAMMA = 2.0
ALPHA_COUNT, ALPHA_HEATMAP, ALPHA_CONF = 1.0, 10.0, 1.5
EPS = 1e-6

F32 = mybir.dt.float32
ALU = mybir.AluOpType
ACTF = mybir.ActivationFunctionType
AX = mybir.AxisListType


def _install_ntff_hook():
    """Provide antenv.axon_hooks if the image lacks it, so that
    run_bass_kernel_spmd(trace=True) (or BASS_TRACE=1) doesn't crash and,
    when possible, actually profiles via the axon .so."""
    try:
        from antenv.axon_hooks import get_axon_ntff_profile_hook  # noqa: F401
        return
    except ImportError:
        pass
    try:
        import antenv
    except ImportError:
        return
    import contextlib
    import ctypes

    mod = types.ModuleType("antenv.axon_hooks")
    _h = [None]
    mod.set_axon_ntff_profile_hook = lambda h: _h.__setitem__(0, h)
    mod.get_axon_ntff_profile_hook = lambda: _h[0]
    sys.modules["antenv.axon_hooks"] = mod
    antenv.axon_hooks = mod

    so_path = "/opt/axon/libaxon_pjrt.so"
    try:
        lib = ctypes.CDLL(so_path)
        if not hasattr(lib, "axon_start_nrt_profile"):
            return
        lib.axon_start_nrt_profile.argtypes = [
            ctypes.POINTER(ctypes.c_int64),
            ctypes.c_size_t,
        ]
        lib.axon_start_nrt_profile.restype = ctypes.c_int64
        lib.axon_stop_nrt_profile.argtypes = [ctypes.c_char_p]
        lib.axon_stop_nrt_profile.restype = ctypes.c_int64
    except OSError:
        return

    @contextlib.contextmanager
    def _hook(output_dir, device_ids):
        import jax

        jax.devices()
        if device_ids:
            ids = (ctypes.c_int64 * len(device_ids))(*device_ids)
            rc = lib.axon_start_nrt_profile(ids, len(device_ids))
        else:
            rc = lib.axon_start_nrt_profile(None, 0)
        if rc != 0:
            raise RuntimeError(f"axon_start_nrt_profile rc={rc}")
        try:
            yield
        finally:
            n = lib.axon_stop_nrt_profile(str(output_dir).encode())
            print(f"profile: {n} file(s) written to {output_dir}", file=sys.stderr)

    mod.set_axon_ntff_profile_hook(_hook)


_install_ntff_hook()

# The axon trace path uploads artifacts to shared storage; degrade to a
# no-op if that infra isn't reachable from this container.
_orig_upload = bass_utils.upload_artifacts


def _safe_upload(tmpdir):
    try:
        return _orig_upload(tmpdir)
    except Exception:
        return tmpdir


bass_utils.upload_artifacts = _safe_upload


def build_module():
    nc = bacc.Bacc("TRN2", target_bir_lowering=False, debug=False)

    ph = nc.dram_tensor("ph", [FULL_TILES * 128, COLS], F32, kind="ExternalInput")
    gh = nc.dram_tensor("gh", [FULL_TILES * 128, COLS], F32, kind="ExternalInput")
    pht = nc.dram_tensor("pht", [2 * REM, COLS // 2], F32, kind="ExternalInput")
    ght = nc.dram_tensor("ght", [2 * REM, COLS // 2], F32, kind="ExternalInput")
    cl = nc.dram_tensor("cl", [B_LOC, P + 1], F32, kind="ExternalInput")
    oh = nc.dram_tensor("oh", [B_LOC, P + 1], F32, kind="ExternalInput")
    conf = nc.dram_tensor("conf", [B_LOC, P], F32, kind="ExternalInput")
    tgt = nc.dram_tensor("tgt", [B_LOC, P], F32, kind="ExternalInput")

    out_s1 = nc.dram_tensor("out_s1", [128, NACC], F32, kind="ExternalOutput")
    out_s2 = nc.dram_tensor("out_s2", [128, NACC], F32, kind="ExternalOutput")
    out_ce = nc.dram_tensor("out_ce", [B_LOC, 2], F32, kind="ExternalOutput")
    out_fo = nc.dram_tensor("out_fo", [B_LOC, 1], F32, kind="ExternalOutput")

    with tile.TileContext(nc) as tc:
        with (
            tc.tile_pool(name="bigio", bufs=3) as bigio,
            tc.tile_pool(name="work", bufs=3) as work,
            tc.tile_pool(name="acc", bufs=1) as accp,
            tc.tile_pool(name="small", bufs=1) as small,
        ):
            sums_sq = accp.tile([128, NACC], F32, tag="ssq")
            sums_st = accp.tile([128, NACC], F32, tag="sst")
            nc.gpsimd.memset(sums_sq[:], 0.0)
            nc.gpsimd.memset(sums_st[:], 0.0)

            # ---- small losses, part 1 (everything except the Ln's) ----
            # count cross-entropy pieces
            cl_t = small.tile([B_LOC, P + 1], F32, tag="cl")
            oh_t = small.tile([B_LOC, P + 1], F32, tag="oh")
            nc.sync.dma_start(cl_t[:], cl[:, :])
            nc.sync.dma_start(oh_t[:], oh[:, :])
            mx = small.tile([B_LOC, 1], F32, tag="mx")
            nc.vector.tensor_reduce(mx[:], cl_t[:], axis=AX.X, op=ALU.max)
            nmx = small.tile([B_LOC, 1], F32, tag="nmx")
            nc.vector.tensor_scalar_mul(nmx[:], mx[:], -1.0)
            et = small.tile([B_LOC, P + 1], F32, tag="et")
            se = small.tile([B_LOC, 1], F32, tag="se")
            nc.scalar.activation(
                et[:], cl_t[:], ACTF.Exp, bias=nmx[:], scale=1.0, accum_out=se[:]
            )
            junk21 = small.tile([B_LOC, P + 1], F32, tag="junk21")
            tg = small.tile([B_LOC, 1], F32, tag="tg")
            nc.vector.scalar_tensor_tensor(
                out=junk21[:], in0=cl_t[:], scalar=1.0, in1=oh_t[:],
                op0=ALU.mult, op1=ALU.mult, accum_out=tg[:],
            )
            pre = small.tile([B_LOC, 1], F32, tag="pre")
            nc.vector.tensor_sub(pre[:], mx[:], tg[:])

            # focal: p_t = 1 - |t - sigma(l)| with sigma from exp(-|l|)
            lt_ = small.tile([B_LOC, P], F32, tag="lt")
            tt_ = small.tile([B_LOC, P], F32, tag="tt")
            nc.sync.dma_start(lt_[:], conf[:, :])
            nc.sync.dma_start(tt_[:], tgt[:, :])
            ab = small.tile([B_LOC, P], F32, tag="ab")
            nc.vector.scalar_tensor_tensor(
                out=ab[:], in0=lt_[:], scalar=-1.0, in1=lt_[:],
                op0=ALU.mult, op1=ALU.max,
            )
            z = small.tile([B_LOC, P], F32, tag="z")
            nc.scalar.activation(z[:], ab[:], ACTF.Exp, scale=-1.0)
            zz = small.tile([B_LOC, P], F32, tag="zz")
            nc.vector.tensor_scalar(zz[:], z[:], 1.0, None, op0=ALU.add)
            r = small.tile([B_LOC, P], F32, tag="r")
            nc.vector.reciprocal(r[:], zz[:])          # sigma(|l|)
            sgn = small.tile([B_LOC, P], F32, tag="sgn")
            nc.vector.tensor_scalar(sgn[:], lt_[:], 0.0, None, op0=ALU.is_ge)
            t1 = small.tile([B_LOC, P], F32, tag="t1")
            nc.vector.tensor_scalar(t1[:], r[:], 2.0, -1.0, op0=ALU.mult, op1=ALU.add)
            t2 = small.tile([B_LOC, P], F32, tag="t2")
            nc.vector.tensor_scalar(t2[:], r[:], -1.0, 1.0, op0=ALU.mult, op1=ALU.add)
            sl0 = small.tile([B_LOC, P], F32, tag="sl0")
            nc.vector.scalar_tensor_tensor(
                out=sl0[:], in0=sgn[:], scalar=1.0, in1=t1[:],
                op0=ALU.mult, op1=ALU.mult,
            )
            sig = small.tile([B_LOC, P], F32, tag="sig")
            nc.vector.tensor_add(sig[:], sl0[:], t2[:])
            u = small.tile([B_LOC, P], F32, tag="u")
            nc.vector.tensor_sub(u[:], tt_[:], sig[:])
            au = small.tile([B_LOC, P], F32, tag="au")
            nc.vector.scalar_tensor_tensor(
                out=au[:], in0=u[:], scalar=-1.0, in1=u[:],
                op0=ALU.mult, op1=ALU.max,
            )
            pt = small.tile([B_LOC, P], F32, tag="pt")
            nc.vector.tensor_scalar(pt[:], au[:], -1.0, 1.0, op0=ALU.mult, op1=ALU.add)
            au2 = small.tile([B_LOC, P], F32, tag="au2")
            nc.vector.tensor_mul(au2[:], au[:], au[:])

            # ---- heavy loop: forward-only DVE -> ACT pipeline ----
            # Full-row chunks [128, 4096]: per-partition 16 KB contiguous
            # DMA runs. Last 40 rows come host-folded as [80, 2048].
            # The ACT squares run in-place (no extra tiles): WAR edges are
            # forward-only (DVE mask reads d/g before ACT overwrites).
            for idx in range(NACC):
                tail = idx == FULL_TILES
                rr = 2 * REM if tail else 128
                cc = COLS // 2 if tail else COLS
                pt_ = bigio.tile([128, COLS], F32, tag="p")
                gt_ = bigio.tile([128, COLS], F32, tag="g")
                dt_ = work.tile([128, COLS], F32, tag="d")
                if tail:
                    nc.sync.dma_start(pt_[:rr, :cc], pht[:, :])
                    nc.sync.dma_start(gt_[:rr, :cc], ght[:, :])
                else:
                    rs = slice(idx * 128, (idx + 1) * 128)
                    nc.sync.dma_start(pt_[:, :], ph[rs, :])
                    nc.sync.dma_start(gt_[:, :], gh[rs, :])
                # d = p - g
                nc.vector.tensor_sub(dt_[:rr, :cc], pt_[:rr, :cc], gt_[:rr, :cc])
                # m = (g > thresh) * d   (overwrites p's slot)
                nc.vector.scalar_tensor_tensor(
                    out=pt_[:rr, :cc], in0=gt_[:rr, :cc],
                    scalar=float(PEAK_THRESH), in1=dt_[:rr, :cc],
                    op0=ALU.is_gt, op1=ALU.mult,
                )
                # sums_sq[:, idx] = rowsum(d^2)   (in-place square)
                nc.scalar.activation(
                    dt_[:rr, :cc], dt_[:rr, :cc], ACTF.Square,
                    accum_out=sums_sq[:rr, idx : idx + 1],
                )
                # sums_st[:, idx] = rowsum(m^2) (= rowsum(d^2 * step))
                nc.scalar.activation(
                    pt_[:rr, :cc], pt_[:rr, :cc], ACTF.Square,
                    accum_out=sums_st[:rr, idx : idx + 1],
                )

            # ---- small losses, part 2: the Ln's ----
            lnz = small.tile([B_LOC, 1], F32, tag="lnz")
            nc.scalar.activation(lnz[:], se[:], ACTF.Ln)
            cer = small.tile([B_LOC, 2], F32, tag="cer")
            nc.vector.tensor_copy(cer[:, 0:1], pre[:])
            nc.vector.tensor_copy(cer[:, 1:2], lnz[:])
            nc.sync.dma_start(out_ce[:, :], cer[:])

            lnpt = small.tile([B_LOC, P], F32, tag="lnpt")
            nc.scalar.activation(lnpt[:], pt[:], ACTF.Ln)
            junk20 = small.tile([B_LOC, P], F32, tag="junk20")
            fr = small.tile([B_LOC, 1], F32, tag="fr")
            # accum = sum(au^2 * ln(p_t)) = -focal_sum   (host negates)
            nc.vector.scalar_tensor_tensor(
                out=junk20[:], in0=au2[:], scalar=1.0, in1=lnpt[:],
                op0=ALU.mult, op1=ALU.mult, accum_out=fr[:],
            )
            nc.sync.dma_start(out_fo[:, :], fr[:])

            # ---- ship raw heatmap partial sums ----
            nc.sync.dma_start(out_s1[:, :], sums_sq[:])
            nc.sync.dma_start(out_s2[:, :], sums_st[:])

    nc.compile()
    return nc


_MODULE = None


def _module():
    global _MODULE
    if _MODULE is None:
        _MODULE = build_module()
    return _MODULE


def _fold_tail(flat):
    """Last REM rows of [680, 4096] -> [2*REM, 2048]: partition
    q = h*REM + r <-> row 640+r, column half h."""
    rest = flat[FULL_TILES * 128 :].reshape(REM, 2, COLS // 2)  # r, h, x
    return np.ascontiguousarray(
        rest.transpose(1, 0, 2).reshape(2 * REM, COLS // 2)
    )


def make_in_maps(count_logits, pred_heatmaps, pred_conf_logits, gt_heatmaps,
                 count, mask):
    count_logits = np.asarray(count_logits, np.float32)
    pred_heatmaps = np.asarray(pred_heatmaps, np.float32)
    pred_conf_logits = np.asarray(pred_conf_logits, np.float32)
    gt_heatmaps = np.asarray(gt_heatmaps, np.float32)
    count = np.asarray(count, np.int32)
    mask = np.asarray(mask, np.int32)

    in_maps = []
    for i in range(N_CORES):
        b0, b1 = i * B_LOC, (i + 1) * B_LOC
        mloc = mask[b0:b1].astype(np.float32)
        ohm = np.zeros((B_LOC, P + 1), np.float32)
        ohm[np.arange(B_LOC), count[b0:b1]] = 1.0
        phl = np.ascontiguousarray(pred_heatmaps[b0:b1].reshape(ROWS, COLS))
        ghl = np.ascontiguousarray(gt_heatmaps[b0:b1].reshape(ROWS, COLS))
        in_maps.append({
            "ph": phl[: FULL_TILES * 128],
            "gh": ghl[: FULL_TILES * 128],
            "pht": _fold_tail(phl),
            "ght": _fold_tail(ghl),
            "cl": np.ascontiguousarray(count_logits[b0:b1]),
            "oh": ohm,
            "conf": np.ascontiguousarray(pred_conf_logits[b0:b1]),
            "tgt": mloc,
        })
    return in_maps


def _rowsums(comb):
    """[128, NACC] per-chunk sums -> [680] per-row sums."""
    rows = np.concatenate(
        [comb[:, :FULL_TILES].T.reshape(-1), np.zeros(REM)]
    )  # row t*128+p at comb[p, t]
    tail = comb[: 2 * REM, FULL_TILES].reshape(2, REM).sum(axis=0)
    rows[FULL_TILES * 128 :] = tail
    return rows


def combine(results, mask):
    mask = np.asarray(mask)
    hm_sum = 0.0
    ce_sum = 0.0
    fo_sum = 0.0
    for i, res in enumerate(results):
        b0, b1 = i * B_LOC, (i + 1) * B_LOC
        s1 = np.asarray(res["out_s1"], np.float64)  # [128, NACC]
        s2 = np.asarray(res["out_s2"], np.float64)
        rowsum = _rowsums(s1 + (PEAK_WEIGHT - 1.0) * s2)
        mrow = np.repeat(mask[b0:b1].astype(np.float64).reshape(-1), K)
        hm_sum += float(rowsum @ mrow)
        ce = np.asarray(res["out_ce"], np.float64)       # [2,2]: pre, ln(se)
        ce_sum += float(ce.sum())
        fo_sum += -float(np.asarray(res["out_fo"], np.float64).sum())
    msum = float(mask.sum())
    hm = hm_sum / (msum * K * H * W + EPS)
    loss_heatmap = hm if msum > 0 else 0.0
    loss_count = ce_sum / B
    loss_conf = fo_sum / (B * P)
    total = (ALPHA_COUNT * loss_count + ALPHA_HEATMAP * loss_heatmap
             + ALPHA_CONF * loss_conf)
    return np.float32(total)


def run(inputs, trace=False, **kwargs):
    """Run on hardware; returns (output_scalar, BassKernelResults)."""
    nc = _module()
    in_maps = make_in_maps(**inputs)
    res = bass_utils.run_bass_kernel_spmd(
        nc, in_maps, core_ids=list(range(N_CORES)), trace=trace, **kwargs
    )
    out = combine(res.results, inputs["mask"])
    return out, res


def kernel(count_logits, pred_heatmaps, pred_conf_logits, gt_heatmaps,
           count, mask):
    out, _ = run(dict(
        count_logits=count_logits, pred_heatmaps=pred_heatmaps,
        pred_conf_logits=pred_conf_logits, gt_heatmaps=gt_heatmaps,
        count=count, mask=mask,
    ))
    return out


# revision 27
# speedup vs baseline: 1.3315x; 1.3315x over previous
"""End2EndPoseLoss on 8 Trainium2 NeuronCores.

Data-parallel over batch: each core handles B_LOC=2 samples.

Heavy part (per core): row-chunk sums over [680, 4096] pred/gt heatmaps.
Uses the identity (pred-gt)^2 * step(gt>thresh) == ((pred-gt)*step)^2 so
the per-chunk dependency graph is forward-only DVE -> ACT:
  DVE: d = p - g            (tensor_tensor)
  DVE: m = step(g>.2) * d   (scalar_tensor_tensor)
  ACT: Square(d)  + row-accumulate -> sums_sq column
  ACT: Square(m)  + row-accumulate -> sums_st column
Raw per-(row-tile, chunk) sums [128, 12] are DMA'd out; the host applies
the 0/1 sample mask per row and the weighted combination (the scalar
"all-reduce" across cores).

Small losses (count CE over [2,21], conf focal over [2,20]) run on-device
too, issued before the heavy loop so their ACT work hides in the DMA
ramp; the two Ln ops run at the end (one activation-table switch).
"""

import sys
import types
import numpy as np

import concourse.bacc as bacc
import concourse.bass as bass  # noqa: F401
import concourse.mybir as mybir
import concourse.tile as tile
from concourse import bass_utils

# Problem constants (hardcoded per contract).
B, P, K, H, W = 16, 20, 17, 64, 64
N_CORES = 8
B_LOC = B // N_CORES            # 2
ROWS = B_LOC * P * K            # 680
COLS = H * W                    # 4096
FULL_TILES = ROWS // 128         # 5 row-tiles of 128 full rows
REM = ROWS - FULL_TILES * 128    # 40 leftover rows -> folded [80, 2048]
NACC = FULL_TILES + 1            # 6 accumulator columns

PEAK_THRESH = 0.2
PEAK_WEIGHT = 5.0
FOCAL_GAMMA = 2.0
ALPHA_COUNT, ALPHA_HEATMAP, ALPHA_CONF = 1.0, 10.0, 1.5
EPS = 1e-6

F32 = mybir.dt.float32
F16 = mybir.dt.float16
ALU = mybir.AluOpType
ACTF = mybir.ActivationFunctionType
AX = mybir.AxisListType


def _install_ntff_hook():
    """Provide antenv.axon_hooks if the image lacks it, so that
    run_bass_kernel_spmd(trace=True) (or BASS_TRACE=1) doesn't crash and,
    when possible, actually profiles via the axon .so."""
    try:
        from antenv.axon_hooks import get_axon_ntff_profile_hook  # noqa: F401
        return
    except ImportError:
        pass
    try:
        import antenv
    except ImportError:
        return
    import contextlib
    import ctypes

    mod = types.ModuleType("antenv.axon_hooks")
    _h = [None]
    mod.set_axon_ntff_profile_hook = lambda h: _h.__setitem__(0, h)
    mod.get_axon_ntff_profile_hook = lambda: _h[0]
    sys.modules["antenv.axon_hooks"] = mod
    antenv.axon_hooks = mod

    so_path = "/opt/axon/libaxon_pjrt.so"
    try:
        lib = ctypes.CDLL(so_path)
        if not hasattr(lib, "axon_start_nrt_profile"):
            return
        lib.axon_start_nrt_profile.argtypes = [
            ctypes.POINTER(ctypes.c_int64),
            ctypes.c_size_t,
        ]
        lib.axon_start_nrt_profile.restype = ctypes.c_int64
        lib.axon_stop_nrt_profile.argtypes = [ctypes.c_char_p]
        lib.axon_stop_nrt_profile.restype = ctypes.c_int64
    except OSError:
        return

    @contextlib.contextmanager
    def _hook(output_dir, device_ids):
        import jax

        jax.devices()
        if device_ids:
            ids = (ctypes.c_int64 * len(device_ids))(*device_ids)
            rc = lib.axon_start_nrt_profile(ids, len(device_ids))
        else:
            rc = lib.axon_start_nrt_profile(None, 0)
        if rc != 0:
            raise RuntimeError(f"axon_start_nrt_profile rc={rc}")
        try:
            yield
        finally:
            n = lib.axon_stop_nrt_profile(str(output_dir).encode())
            print(f"profile: {n} file(s) written to {output_dir}", file=sys.stderr)

    mod.set_axon_ntff_profile_hook(_hook)


_install_ntff_hook()

# The axon trace path uploads artifacts to shared storage; degrade to a
# no-op if that infra isn't reachable from this container.
_orig_upload = bass_utils.upload_artifacts


def _safe_upload(tmpdir):
    try:
        return _orig_upload(tmpdir)
    except Exception:
        return tmpdir


bass_utils.upload_artifacts = _safe_upload


def build_module():
    nc = bacc.Bacc("TRN2", target_bir_lowering=False, debug=False)

    ph = nc.dram_tensor("ph", [FULL_TILES * 128, COLS], F16, kind="ExternalInput")
    gh = nc.dram_tensor("gh", [FULL_TILES * 128, COLS], F16, kind="ExternalInput")
    pht = nc.dram_tensor("pht", [2 * REM, COLS // 2], F16, kind="ExternalInput")
    ght = nc.dram_tensor("ght", [2 * REM, COLS // 2], F16, kind="ExternalInput")
    cl = nc.dram_tensor("cl", [B_LOC, P + 1], F32, kind="ExternalInput")
    oh = nc.dram_tensor("oh", [B_LOC, P + 1], F32, kind="ExternalInput")
    conf = nc.dram_tensor("conf", [B_LOC, P], F32, kind="ExternalInput")
    tgt = nc.dram_tensor("tgt", [B_LOC, P], F32, kind="ExternalInput")

    out_s1 = nc.dram_tensor("out_s1", [128, NACC], F32, kind="ExternalOutput")
    out_s2 = nc.dram_tensor("out_s2", [128, NACC], F32, kind="ExternalOutput")
    out_ce = nc.dram_tensor("out_ce", [B_LOC, 2], F32, kind="ExternalOutput")
    out_fo = nc.dram_tensor("out_fo", [B_LOC, 1], F32, kind="ExternalOutput")

    with tile.TileContext(nc) as tc:
        with (
            tc.tile_pool(name="bigio", bufs=3) as bigio,
            tc.tile_pool(name="work", bufs=3) as work,
            tc.tile_pool(name="acc", bufs=1) as accp,
            tc.tile_pool(name="small", bufs=1) as small,
        ):
            sums_sq = accp.tile([128, NACC], F32, tag="ssq")
            sums_st = accp.tile([128, NACC], F32, tag="sst")
            nc.gpsimd.memset(sums_sq[:], 0.0)
            nc.gpsimd.memset(sums_st[:], 0.0)

            # ---- small losses, part 1 (everything except the Ln's) ----
            # count cross-entropy pieces
            cl_t = small.tile([B_LOC, P + 1], F32, tag="cl")
            oh_t = small.tile([B_LOC, P + 1], F32, tag="oh")
            nc.sync.dma_start(cl_t[:], cl[:, :])
            nc.sync.dma_start(oh_t[:], oh[:, :])
            mx = small.tile([B_LOC, 1], F32, tag="mx")
            nc.vector.tensor_reduce(mx[:], cl_t[:], axis=AX.X, op=ALU.max)
            nmx = small.tile([B_LOC, 1], F32, tag="nmx")
            nc.vector.tensor_scalar_mul(nmx[:], mx[:], -1.0)
            et = small.tile([B_LOC, P + 1], F32, tag="et")
            se = small.tile([B_LOC, 1], F32, tag="se")
            nc.scalar.activation(
                et[:], cl_t[:], ACTF.Exp, bias=nmx[:], scale=1.0, accum_out=se[:]
            )
            junk21 = small.tile([B_LOC, P + 1], F32, tag="junk21")
            tg = small.tile([B_LOC, 1], F32, tag="tg")
            nc.vector.scalar_tensor_tensor(
                out=junk21[:], in0=cl_t[:], scalar=1.0, in1=oh_t[:],
                op0=ALU.mult, op1=ALU.mult, accum_out=tg[:],
            )
            pre = small.tile([B_LOC, 1], F32, tag="pre")
            nc.vector.tensor_sub(pre[:], mx[:], tg[:])

            # focal: p_t = 1 - |t - sigma(l)| with sigma from exp(-|l|)
            lt_ = small.tile([B_LOC, P], F32, tag="lt")
            tt_ = small.tile([B_LOC, P], F32, tag="tt")
            nc.sync.dma_start(lt_[:], conf[:, :])
            nc.sync.dma_start(tt_[:], tgt[:, :])
            ab = small.tile([B_LOC, P], F32, tag="ab")
            nc.vector.scalar_tensor_tensor(
                out=ab[:], in0=lt_[:], scalar=-1.0, in1=lt_[:],
                op0=ALU.mult, op1=ALU.max,
            )
            z = small.tile([B_LOC, P], F32, tag="z")
            nc.scalar.activation(z[:], ab[:], ACTF.Exp, scale=-1.0)
            zz = small.tile([B_LOC, P], F32, tag="zz")
            nc.vector.tensor_scalar(zz[:], z[:], 1.0, None, op0=ALU.add)
            r = small.tile([B_LOC, P], F32, tag="r")
            nc.vector.reciprocal(r[:], zz[:])          # sigma(|l|)
            sgn = small.tile([B_LOC, P], F32, tag="sgn")
            nc.vector.tensor_scalar(sgn[:], lt_[:], 0.0, None, op0=ALU.is_ge)
            t1 = small.tile([B_LOC, P], F32, tag="t1")
            nc.vector.tensor_scalar(t1[:], r[:], 2.0, -1.0, op0=ALU.mult, op1=ALU.add)
            t2 = small.tile([B_LOC, P], F32, tag="t2")
            nc.vector.tensor_scalar(t2[:], r[:], -1.0, 1.0, op0=ALU.mult, op1=ALU.add)
            sl0 = small.tile([B_LOC, P], F32, tag="sl0")
            nc.vector.scalar_tensor_tensor(
                out=sl0[:], in0=sgn[:], scalar=1.0, in1=t1[:],
                op0=ALU.mult, op1=ALU.mult,
            )
            sig = small.tile([B_LOC, P], F32, tag="sig")
            nc.vector.tensor_add(sig[:], sl0[:], t2[:])
            u = small.tile([B_LOC, P], F32, tag="u")
            nc.vector.tensor_sub(u[:], tt_[:], sig[:])
            au = small.tile([B_LOC, P], F32, tag="au")
            nc.vector.scalar_tensor_tensor(
                out=au[:], in0=u[:], scalar=-1.0, in1=u[:],
                op0=ALU.mult, op1=ALU.max,
            )
            pt = small.tile([B_LOC, P], F32, tag="pt")
            nc.vector.tensor_scalar(pt[:], au[:], -1.0, 1.0, op0=ALU.mult, op1=ALU.add)
            au2 = small.tile([B_LOC, P], F32, tag="au2")
            nc.vector.tensor_mul(au2[:], au[:], au[:])

            # ---- heavy loop: forward-only DVE -> ACT pipeline (fp16) ----
            # Full-row chunks [128, 4096] fp16: 8 KB contiguous runs per
            # partition. Last 40 rows come host-folded as [80, 2048].
            # DVE 2-src fp16 ops run in 2x mode; to balance engines the
            # d^2 row-accumulation alternates between ACT (even chunks)
            # and DVE (odd chunks, via (d mult 1) mult d with accum).
            for idx in range(NACC):
                tail = idx == FULL_TILES
                rr = 2 * REM if tail else 128
                cc = COLS // 2 if tail else COLS
                pt_ = bigio.tile([128, COLS], F16, tag="p")
                gt_ = bigio.tile([128, COLS], F16, tag="g")
                dt_ = work.tile([128, COLS], F16, tag="d")
                mt_ = work.tile([128, COLS], F16, tag="m")
                if tail:
                    nc.sync.dma_start(pt_[:rr, :cc], pht[:, :])
                    nc.sync.dma_start(gt_[:rr, :cc], ght[:, :])
                else:
                    rs = slice(idx * 128, (idx + 1) * 128)
                    nc.sync.dma_start(pt_[:, :], ph[rs, :])
                    nc.sync.dma_start(gt_[:, :], gh[rs, :])
                # d = p - g
                nc.vector.tensor_sub(dt_[:rr, :cc], pt_[:rr, :cc], gt_[:rr, :cc])
                # m = (g > thresh) * d
                nc.vector.scalar_tensor_tensor(
                    out=mt_[:rr, :cc], in0=gt_[:rr, :cc],
                    scalar=float(PEAK_THRESH), in1=dt_[:rr, :cc],
                    op0=ALU.is_gt, op1=ALU.mult,
                )
                if idx % 2 == 1:
                    # sums_sq[:, idx] = rowsum(d*d) on DVE (junk out -> p)
                    nc.vector.scalar_tensor_tensor(
                        out=pt_[:rr, :cc], in0=dt_[:rr, :cc], scalar=1.0,
                        in1=dt_[:rr, :cc], op0=ALU.mult, op1=ALU.mult,
                        accum_out=sums_sq[:rr, idx : idx + 1],
                    )
                else:
                    # sums_sq[:, idx] = rowsum(d^2) on ACT (in-place square)
                    nc.scalar.activation(
                        dt_[:rr, :cc], dt_[:rr, :cc], ACTF.Square,
                        accum_out=sums_sq[:rr, idx : idx + 1],
                    )
                # sums_st[:, idx] = rowsum(m^2) (= rowsum(d^2 * step))
                nc.scalar.activation(
                    mt_[:rr, :cc], mt_[:rr, :cc], ACTF.Square,
                    accum_out=sums_st[:rr, idx : idx + 1],
                )

            # ---- small losses, part 2: the Ln's ----
            lnz = small.tile([B_LOC, 1], F32, tag="lnz")
            nc.scalar.activation(lnz[:], se[:], ACTF.Ln)
            cer = small.tile([B_LOC, 2], F32, tag="cer")
            nc.vector.tensor_copy(cer[:, 0:1], pre[:])
            nc.vector.tensor_copy(cer[:, 1:2], lnz[:])
            nc.sync.dma_start(out_ce[:, :], cer[:])

            lnpt = small.tile([B_LOC, P], F32, tag="lnpt")
            nc.scalar.activation(lnpt[:], pt[:], ACTF.Ln)
            junk20 = small.tile([B_LOC, P], F32, tag="junk20")
            fr = small.tile([B_LOC, 1], F32, tag="fr")
            # accum = sum(au^2 * ln(p_t)) = -focal_sum   (host negates)
            nc.vector.scalar_tensor_tensor(
                out=junk20[:], in0=au2[:], scalar=1.0, in1=lnpt[:],
                op0=ALU.mult, op1=ALU.mult, accum_out=fr[:],
            )
            nc.sync.dma_start(out_fo[:, :], fr[:])

            # ---- ship raw heatmap partial sums ----
            nc.sync.dma_start(out_s1[:, :], sums_sq[:])
            nc.sync.dma_start(out_s2[:, :], sums_st[:])

    nc.compile()
    return nc


_MODULE = None


def _module():
    global _MODULE
    if _MODULE is None:
        _MODULE = build_module()
    return _MODULE


def _fold_tail(flat):
    """Last REM rows of [680, 4096] -> [2*REM, 2048]: partition
    q = h*REM + r <-> row 640+r, column half h."""
    rest = flat[FULL_TILES * 128 :].reshape(REM, 2, COLS // 2)  # r, h, x
    return np.ascontiguousarray(
        rest.transpose(1, 0, 2).reshape(2 * REM, COLS // 2)
    )


def make_in_maps(count_logits, pred_heatmaps, pred_conf_logits, gt_heatmaps,
                 count, mask):
    count_logits = np.asarray(count_logits, np.float32)
    pred_heatmaps = np.asarray(pred_heatmaps, np.float32)
    pred_conf_logits = np.asarray(pred_conf_logits, np.float32)
    gt_heatmaps = np.asarray(gt_heatmaps, np.float32)
    count = np.asarray(count, np.int32)
    mask = np.asarray(mask, np.int32)

    in_maps = []
    for i in range(N_CORES):
        b0, b1 = i * B_LOC, (i + 1) * B_LOC
        mloc = mask[b0:b1].astype(np.float32)
        ohm = np.zeros((B_LOC, P + 1), np.float32)
        ohm[np.arange(B_LOC), count[b0:b1]] = 1.0
        phl = pred_heatmaps[b0:b1].reshape(ROWS, COLS).astype(np.float16)
        ghl = gt_heatmaps[b0:b1].reshape(ROWS, COLS).astype(np.float16)
        in_maps.append({
            "ph": np.ascontiguousarray(phl[: FULL_TILES * 128]),
            "gh": np.ascontiguousarray(ghl[: FULL_TILES * 128]),
            "pht": _fold_tail(phl),
            "ght": _fold_tail(ghl),
            "cl": np.ascontiguousarray(count_logits[b0:b1]),
            "oh": ohm,
            "conf": np.ascontiguousarray(pred_conf_logits[b0:b1]),
            "tgt": mloc,
        })
    return in_maps


def _rowsums(comb):
    """[128, NACC] per-chunk sums -> [680] per-row sums."""
    rows = np.concatenate(
        [comb[:, :FULL_TILES].T.reshape(-1), np.zeros(REM)]
    )  # row t*128+p at comb[p, t]
    tail = comb[: 2 * REM, FULL_TILES].reshape(2, REM).sum(axis=0)
    rows[FULL_TILES * 128 :] = tail
    return rows


def combine(results, mask):
    mask = np.asarray(mask)
    hm_sum = 0.0
    ce_sum = 0.0
    fo_sum = 0.0
    for i, res in enumerate(results):
        b0, b1 = i * B_LOC, (i + 1) * B_LOC
        s1 = np.asarray(res["out_s1"], np.float64)  # [128, NACC]
        s2 = np.asarray(res["out_s2"], np.float64)
        rowsum = _rowsums(s1 + (PEAK_WEIGHT - 1.0) * s2)
        mrow = np.repeat(mask[b0:b1].astype(np.float64).reshape(-1), K)
        hm_sum += float(rowsum @ mrow)
        ce = np.asarray(res["out_ce"], np.float64)       # [2,2]: pre, ln(se)
        ce_sum += float(ce.sum())
        fo_sum += -float(np.asarray(res["out_fo"], np.float64).sum())
    msum = float(mask.sum())
    hm = hm_sum / (msum * K * H * W + EPS)
    loss_heatmap = hm if msum > 0 else 0.0
    loss_count = ce_sum / B
    loss_conf = fo_sum / (B * P)
    total = (ALPHA_COUNT * loss_count + ALPHA_HEATMAP * loss_heatmap
             + ALPHA_CONF * loss_conf)
    return np.float32(total)


def run(inputs, trace=False, **kwargs):
    """Run on hardware; returns (output_scalar, BassKernelResults)."""
    nc = _module()
    in_maps = make_in_maps(**inputs)
    res = bass_utils.run_bass_kernel_spmd(
        nc, in_maps, core_ids=list(range(N_CORES)), trace=trace, **kwargs
    )
    out = combine(res.results, inputs["mask"])
    return out, res


def kernel(count_logits, pred_heatmaps, pred_conf_logits, gt_heatmaps,
           count, mask):
    out, _ = run(dict(
        count_logits=count_logits, pred_heatmaps=pred_heatmaps,
        pred_conf_logits=pred_conf_logits, gt_heatmaps=gt_heatmaps,
        count=count, mask=mask,
    ))
    return out


# revision 28
# speedup vs baseline: 1.5438x; 1.1595x over previous
"""End2EndPoseLoss on 8 Trainium2 NeuronCores.

Data-parallel over batch: each core handles B_LOC=2 samples.

Heavy part (per core): row-chunk sums over [680, 4096] pred/gt heatmaps.
Uses the identity (pred-gt)^2 * step(gt>thresh) == ((pred-gt)*step)^2 so
the per-chunk dependency graph is forward-only DVE -> ACT:
  DVE: d = p - g            (tensor_tensor)
  DVE: m = step(g>.2) * d   (scalar_tensor_tensor)
  ACT: Square(d)  + row-accumulate -> sums_sq column
  ACT: Square(m)  + row-accumulate -> sums_st column
Raw per-(row-tile, chunk) sums [128, 12] are DMA'd out; the host applies
the 0/1 sample mask per row and the weighted combination (the scalar
"all-reduce" across cores).

Small losses (count CE over [2,21], conf focal over [2,20]) run on-device
too, issued before the heavy loop so their ACT work hides in the DMA
ramp; the two Ln ops run at the end (one activation-table switch).
"""

import sys
import types
import numpy as np

import concourse.bacc as bacc
import concourse.bass as bass  # noqa: F401
import concourse.mybir as mybir
import concourse.tile as tile
from concourse import bass_utils

# Problem constants (hardcoded per contract).
B, P, K, H, W = 16, 20, 17, 64, 64
N_CORES = 8
B_LOC = B // N_CORES            # 2
ROWS = B_LOC * P * K            # 680
COLS = H * W                    # 4096
FULL_TILES = ROWS // 128         # 5 row-tiles of 128 full rows
REM = ROWS - FULL_TILES * 128    # 40 leftover rows -> folded [80, 2048]
NACC = FULL_TILES + 1            # 6 accumulator columns

PEAK_THRESH = 0.2
PEAK_WEIGHT = 5.0
FOCAL_GAMMA = 2.0
ALPHA_COUNT, ALPHA_HEATMAP, ALPHA_CONF = 1.0, 10.0, 1.5
EPS = 1e-6

F32 = mybir.dt.float32
F16 = mybir.dt.float16
ALU = mybir.AluOpType
ACTF = mybir.ActivationFunctionType
AX = mybir.AxisListType


def _install_ntff_hook():
    """Provide antenv.axon_hooks if the image lacks it, so that
    run_bass_kernel_spmd(trace=True) (or BASS_TRACE=1) doesn't crash and,
    when possible, actually profiles via the axon .so."""
    try:
        from antenv.axon_hooks import get_axon_ntff_profile_hook  # noqa: F401
        return
    except ImportError:
        pass
    try:
        import antenv
    except ImportError:
        return
    import contextlib
    import ctypes

    mod = types.ModuleType("antenv.axon_hooks")
    _h = [None]
    mod.set_axon_ntff_profile_hook = lambda h: _h.__setitem__(0, h)
    mod.get_axon_ntff_profile_hook = lambda: _h[0]
    sys.modules["antenv.axon_hooks"] = mod
    antenv.axon_hooks = mod

    so_path = "/opt/axon/libaxon_pjrt.so"
    try:
        lib = ctypes.CDLL(so_path)
        if not hasattr(lib, "axon_start_nrt_profile"):
            return
        lib.axon_start_nrt_profile.argtypes = [
            ctypes.POINTER(ctypes.c_int64),
            ctypes.c_size_t,
        ]
        lib.axon_start_nrt_profile.restype = ctypes.c_int64
        lib.axon_stop_nrt_profile.argtypes = [ctypes.c_char_p]
        lib.axon_stop_nrt_profile.restype = ctypes.c_int64
    except OSError:
        return

    @contextlib.contextmanager
    def _hook(output_dir, device_ids):
        import jax

        jax.devices()
        if device_ids:
            ids = (ctypes.c_int64 * len(device_ids))(*device_ids)
            rc = lib.axon_start_nrt_profile(ids, len(device_ids))
        else:
            rc = lib.axon_start_nrt_profile(None, 0)
        if rc != 0:
            raise RuntimeError(f"axon_start_nrt_profile rc={rc}")
        try:
            yield
        finally:
            n = lib.axon_stop_nrt_profile(str(output_dir).encode())
            print(f"profile: {n} file(s) written to {output_dir}", file=sys.stderr)

    mod.set_axon_ntff_profile_hook(_hook)


_install_ntff_hook()

# The axon trace path uploads artifacts to shared storage; degrade to a
# no-op if that infra isn't reachable from this container.
_orig_upload = bass_utils.upload_artifacts


def _safe_upload(tmpdir):
    try:
        return _orig_upload(tmpdir)
    except Exception:
        return tmpdir


bass_utils.upload_artifacts = _safe_upload


def build_module():
    nc = bacc.Bacc("TRN2", target_bir_lowering=False, debug=False)

    ph = nc.dram_tensor("ph", [FULL_TILES * 128, COLS], F16, kind="ExternalInput")
    gh = nc.dram_tensor("gh", [FULL_TILES * 128, COLS], F16, kind="ExternalInput")
    pht = nc.dram_tensor("pht", [2 * REM, COLS // 2], F16, kind="ExternalInput")
    ght = nc.dram_tensor("ght", [2 * REM, COLS // 2], F16, kind="ExternalInput")
    cl = nc.dram_tensor("cl", [B_LOC, P + 1], F32, kind="ExternalInput")
    oh = nc.dram_tensor("oh", [B_LOC, P + 1], F32, kind="ExternalInput")
    conf = nc.dram_tensor("conf", [B_LOC, P], F32, kind="ExternalInput")
    tgt = nc.dram_tensor("tgt", [B_LOC, P], F32, kind="ExternalInput")

    out_s1 = nc.dram_tensor("out_s1", [128, NACC], F32, kind="ExternalOutput")
    out_s2 = nc.dram_tensor("out_s2", [128, NACC], F32, kind="ExternalOutput")
    out_ce = nc.dram_tensor("out_ce", [B_LOC, 2], F32, kind="ExternalOutput")
    out_fo = nc.dram_tensor("out_fo", [B_LOC, 1], F32, kind="ExternalOutput")

    with tile.TileContext(nc) as tc:
        with (
            tc.tile_pool(name="bigio", bufs=3) as bigio,
            tc.tile_pool(name="work", bufs=3) as work,
            tc.tile_pool(name="acc", bufs=1) as accp,
            tc.tile_pool(name="small", bufs=1) as small,
        ):
            sums_sq = accp.tile([128, NACC], F32, tag="ssq")
            sums_st = accp.tile([128, NACC], F32, tag="sst")
            nc.gpsimd.memset(sums_sq[:], 0.0)
            nc.gpsimd.memset(sums_st[:], 0.0)

            # ---- small losses, part 1 (everything except the Ln's) ----
            # count cross-entropy pieces
            cl_t = small.tile([B_LOC, P + 1], F32, tag="cl")
            oh_t = small.tile([B_LOC, P + 1], F32, tag="oh")
            nc.sync.dma_start(cl_t[:], cl[:, :])
            nc.sync.dma_start(oh_t[:], oh[:, :])
            mx = small.tile([B_LOC, 1], F32, tag="mx")
            nc.vector.tensor_reduce(mx[:], cl_t[:], axis=AX.X, op=ALU.max)
            nmx = small.tile([B_LOC, 1], F32, tag="nmx")
            nc.vector.tensor_scalar_mul(nmx[:], mx[:], -1.0)
            et = small.tile([B_LOC, P + 1], F32, tag="et")
            se = small.tile([B_LOC, 1], F32, tag="se")
            nc.scalar.activation(
                et[:], cl_t[:], ACTF.Exp, bias=nmx[:], scale=1.0, accum_out=se[:]
            )
            junk21 = small.tile([B_LOC, P + 1], F32, tag="junk21")
            tg = small.tile([B_LOC, 1], F32, tag="tg")
            nc.vector.scalar_tensor_tensor(
                out=junk21[:], in0=cl_t[:], scalar=1.0, in1=oh_t[:],
                op0=ALU.mult, op1=ALU.mult, accum_out=tg[:],
            )
            pre = small.tile([B_LOC, 1], F32, tag="pre")
            nc.vector.tensor_sub(pre[:], mx[:], tg[:])

            # focal: p_t = 1 - |t - sigma(l)| with sigma from exp(-|l|)
            lt_ = small.tile([B_LOC, P], F32, tag="lt")
            tt_ = small.tile([B_LOC, P], F32, tag="tt")
            nc.sync.dma_start(lt_[:], conf[:, :])
            nc.sync.dma_start(tt_[:], tgt[:, :])
            ab = small.tile([B_LOC, P], F32, tag="ab")
            nc.vector.scalar_tensor_tensor(
                out=ab[:], in0=lt_[:], scalar=-1.0, in1=lt_[:],
                op0=ALU.mult, op1=ALU.max,
            )
            z = small.tile([B_LOC, P], F32, tag="z")
            nc.scalar.activation(z[:], ab[:], ACTF.Exp, scale=-1.0)
            zz = small.tile([B_LOC, P], F32, tag="zz")
            nc.vector.tensor_scalar(zz[:], z[:], 1.0, None, op0=ALU.add)
            r = small.tile([B_LOC, P], F32, tag="r")
            nc.vector.reciprocal(r[:], zz[:])          # sigma(|l|)
            sgn = small.tile([B_LOC, P], F32, tag="sgn")
            nc.vector.tensor_scalar(sgn[:], lt_[:], 0.0, None, op0=ALU.is_ge)
            t1 = small.tile([B_LOC, P], F32, tag="t1")
            nc.vector.tensor_scalar(t1[:], r[:], 2.0, -1.0, op0=ALU.mult, op1=ALU.add)
            t2 = small.tile([B_LOC, P], F32, tag="t2")
            nc.vector.tensor_scalar(t2[:], r[:], -1.0, 1.0, op0=ALU.mult, op1=ALU.add)
            sl0 = small.tile([B_LOC, P], F32, tag="sl0")
            nc.vector.scalar_tensor_tensor(
                out=sl0[:], in0=sgn[:], scalar=1.0, in1=t1[:],
                op0=ALU.mult, op1=ALU.mult,
            )
            sig = small.tile([B_LOC, P], F32, tag="sig")
            nc.vector.tensor_add(sig[:], sl0[:], t2[:])
            u = small.tile([B_LOC, P], F32, tag="u")
            nc.vector.tensor_sub(u[:], tt_[:], sig[:])
            au = small.tile([B_LOC, P], F32, tag="au")
            nc.vector.scalar_tensor_tensor(
                out=au[:], in0=u[:], scalar=-1.0, in1=u[:],
                op0=ALU.mult, op1=ALU.max,
            )
            pt = small.tile([B_LOC, P], F32, tag="pt")
            nc.vector.tensor_scalar(pt[:], au[:], -1.0, 1.0, op0=ALU.mult, op1=ALU.add)
            au2 = small.tile([B_LOC, P], F32, tag="au2")
            nc.vector.tensor_mul(au2[:], au[:], au[:])

            # ---- heavy loop: forward-only DVE -> ACT pipeline (fp16) ----
            # Full-row chunks [128, 4096] fp16: 8 KB contiguous runs per
            # partition. Last 40 rows come host-folded as [80, 2048].
            # DVE fp16 perf modes: tensor_scalar cmp 4x, tensor_tensor 2x
            # (scalar_tensor_tensor only has 1x uops - avoided for the
            # mask). The d^2 row-accumulation goes to ACT on most chunks
            # and to DVE (1x stt) on the rest to balance the engines.
            DVE_SQ = {4, FULL_TILES}
            for idx in range(NACC):
                tail = idx == FULL_TILES
                rr = 2 * REM if tail else 128
                cc = COLS // 2 if tail else COLS
                pt_ = bigio.tile([128, COLS], F16, tag="p")
                gt_ = bigio.tile([128, COLS], F16, tag="g")
                dt_ = work.tile([128, COLS], F16, tag="d")
                mt_ = work.tile([128, COLS], F16, tag="m")
                st_ = work.tile([128, COLS], F16, tag="s")
                if tail:
                    nc.sync.dma_start(pt_[:rr, :cc], pht[:, :])
                    nc.sync.dma_start(gt_[:rr, :cc], ght[:, :])
                else:
                    rs = slice(idx * 128, (idx + 1) * 128)
                    nc.sync.dma_start(pt_[:, :], ph[rs, :])
                    nc.sync.dma_start(gt_[:, :], gh[rs, :])
                # d = p - g                      (TT, 2x)
                nc.vector.tensor_sub(dt_[:rr, :cc], pt_[:rr, :cc], gt_[:rr, :cc])
                # s = (g > thresh)               (TS cmp, 4x)
                nc.vector.tensor_scalar(
                    st_[:rr, :cc], gt_[:rr, :cc], float(PEAK_THRESH), None,
                    op0=ALU.is_gt,
                )
                # m = s * d                      (TT, 2x)
                nc.vector.tensor_mul(mt_[:rr, :cc], st_[:rr, :cc], dt_[:rr, :cc])
                if idx in DVE_SQ:
                    # sums_sq[:, idx] = rowsum(d*d) on DVE (junk out -> p)
                    nc.vector.scalar_tensor_tensor(
                        out=pt_[:rr, :cc], in0=dt_[:rr, :cc], scalar=1.0,
                        in1=dt_[:rr, :cc], op0=ALU.mult, op1=ALU.mult,
                        accum_out=sums_sq[:rr, idx : idx + 1],
                    )
                else:
                    # sums_sq[:, idx] = rowsum(d^2) on ACT (in-place square)
                    nc.scalar.activation(
                        dt_[:rr, :cc], dt_[:rr, :cc], ACTF.Square,
                        accum_out=sums_sq[:rr, idx : idx + 1],
                    )
                # sums_st[:, idx] = rowsum(m^2) (= rowsum(d^2 * step))
                nc.scalar.activation(
                    mt_[:rr, :cc], mt_[:rr, :cc], ACTF.Square,
                    accum_out=sums_st[:rr, idx : idx + 1],
                )

            # ---- small losses, part 2: the Ln's ----
            lnz = small.tile([B_LOC, 1], F32, tag="lnz")
            nc.scalar.activation(lnz[:], se[:], ACTF.Ln)
            cer = small.tile([B_LOC, 2], F32, tag="cer")
            nc.vector.tensor_copy(cer[:, 0:1], pre[:])
            nc.vector.tensor_copy(cer[:, 1:2], lnz[:])
            nc.sync.dma_start(out_ce[:, :], cer[:])

            lnpt = small.tile([B_LOC, P], F32, tag="lnpt")
            nc.scalar.activation(lnpt[:], pt[:], ACTF.Ln)
            junk20 = small.tile([B_LOC, P], F32, tag="junk20")
            fr = small.tile([B_LOC, 1], F32, tag="fr")
            # accum = sum(au^2 * ln(p_t)) = -focal_sum   (host negates)
            nc.vector.scalar_tensor_tensor(
                out=junk20[:], in0=au2[:], scalar=1.0, in1=lnpt[:],
                op0=ALU.mult, op1=ALU.mult, accum_out=fr[:],
            )
            nc.sync.dma_start(out_fo[:, :], fr[:])

            # ---- ship raw heatmap partial sums ----
            nc.sync.dma_start(out_s1[:, :], sums_sq[:])
            nc.sync.dma_start(out_s2[:, :], sums_st[:])

    nc.compile()
    return nc


_MODULE = None


def _module():
    global _MODULE
    if _MODULE is None:
        _MODULE = build_module()
    return _MODULE


def _fold_tail(flat):
    """Last REM rows of [680, 4096] -> [2*REM, 2048]: partition
    q = h*REM + r <-> row 640+r, column half h."""
    rest = flat[FULL_TILES * 128 :].reshape(REM, 2, COLS // 2)  # r, h, x
    return np.ascontiguousarray(
        rest.transpose(1, 0, 2).reshape(2 * REM, COLS // 2)
    )


def make_in_maps(count_logits, pred_heatmaps, pred_conf_logits, gt_heatmaps,
                 count, mask):
    count_logits = np.asarray(count_logits, np.float32)
    pred_heatmaps = np.asarray(pred_heatmaps, np.float32)
    pred_conf_logits = np.asarray(pred_conf_logits, np.float32)
    gt_heatmaps = np.asarray(gt_heatmaps, np.float32)
    count = np.asarray(count, np.int32)
    mask = np.asarray(mask, np.int32)

    in_maps = []
    for i in range(N_CORES):
        b0, b1 = i * B_LOC, (i + 1) * B_LOC
        mloc = mask[b0:b1].astype(np.float32)
        ohm = np.zeros((B_LOC, P + 1), np.float32)
        ohm[np.arange(B_LOC), count[b0:b1]] = 1.0
        phl = pred_heatmaps[b0:b1].reshape(ROWS, COLS).astype(np.float16)
        ghl = gt_heatmaps[b0:b1].reshape(ROWS, COLS).astype(np.float16)
        in_maps.append({
            "ph": np.ascontiguousarray(phl[: FULL_TILES * 128]),
            "gh": np.ascontiguousarray(ghl[: FULL_TILES * 128]),
            "pht": _fold_tail(phl),
            "ght": _fold_tail(ghl),
            "cl": np.ascontiguousarray(count_logits[b0:b1]),
            "oh": ohm,
            "conf": np.ascontiguousarray(pred_conf_logits[b0:b1]),
            "tgt": mloc,
        })
    return in_maps


def _rowsums(comb):
    """[128, NACC] per-chunk sums -> [680] per-row sums."""
    rows = np.concatenate(
        [comb[:, :FULL_TILES].T.reshape(-1), np.zeros(REM)]
    )  # row t*128+p at comb[p, t]
    tail = comb[: 2 * REM, FULL_TILES].reshape(2, REM).sum(axis=0)
    rows[FULL_TILES * 128 :] = tail
    return rows


def combine(results, mask):
    mask = np.asarray(mask)
    hm_sum = 0.0
    ce_sum = 0.0
    fo_sum = 0.0
    for i, res in enumerate(results):
        b0, b1 = i * B_LOC, (i + 1) * B_LOC
        s1 = np.asarray(res["out_s1"], np.float64)  # [128, NACC]
        s2 = np.asarray(res["out_s2"], np.float64)
        rowsum = _rowsums(s1 + (PEAK_WEIGHT - 1.0) * s2)
        mrow = np.repeat(mask[b0:b1].astype(np.float64).reshape(-1), K)
        hm_sum += float(rowsum @ mrow)
        ce = np.asarray(res["out_ce"], np.float64)       # [2,2]: pre, ln(se)
        ce_sum += float(ce.sum())
        fo_sum += -float(np.asarray(res["out_fo"], np.float64).sum())
    msum = float(mask.sum())
    hm = hm_sum / (msum * K * H * W + EPS)
    loss_heatmap = hm if msum > 0 else 0.0
    loss_count = ce_sum / B
    loss_conf = fo_sum / (B * P)
    total = (ALPHA_COUNT * loss_count + ALPHA_HEATMAP * loss_heatmap
             + ALPHA_CONF * loss_conf)
    return np.float32(total)


def run(inputs, trace=False, **kwargs):
    """Run on hardware; returns (output_scalar, BassKernelResults)."""
    nc = _module()
    in_maps = make_in_maps(**inputs)
    res = bass_utils.run_bass_kernel_spmd(
        nc, in_maps, core_ids=list(range(N_CORES)), trace=trace, **kwargs
    )
    out = combine(res.results, inputs["mask"])
    return out, res


def kernel(count_logits, pred_heatmaps, pred_conf_logits, gt_heatmaps,
           count, mask):
    out, _ = run(dict(
        count_logits=count_logits, pred_heatmaps=pred_heatmaps,
        pred_conf_logits=pred_conf_logits, gt_heatmaps=gt_heatmaps,
        count=count, mask=mask,
    ))
    return out
